# revision 14
# baseline (speedup 1.0000x reference)
"""MiniMax-M2 MoE kernel for 8 Trainium2 NeuronCores.

Strategy (expert-parallel, sparse/routed):
  Host: router gate matmul + sigmoid + top-4 selection + combine-weight
    renormalization in f32 numpy (pure data movement / tiny matmul), then
    gather tokens per expert, pad each expert slot to a static capacity.
  Host: quantize activations and weights to fp8-e4m3 hi/lo pairs
    (value = hi + lo exactly to ~2^-14 relative) so the device can run all
    matmuls in DoubleRow fp8 perf mode while keeping ~bf16 accuracy via
    3-product error compensation (hi*hi + hi*lo + lo*hi; lo*lo dropped).
  Device (expert-parallel): per core, 2 expert slots' SwiGLU FFN over the
    gathered tokens; combine weight applied on device; bf16 outputs.
  Host: scatter-add per-expert outputs into the [T, H] result in expert
    order (matches the reference scan accumulation order).

Scale bookkeeping (per-tensor power-of-2 scales, folded into constants):
  x*SX, w1*SW1, w3*SW3, w2*SW2 quantized to fp8 hi+lo.
  PSUM_g = SX*SW1 * g      -> silu input scale 1/(SX*SW1)
  PSUM_u = SX*SW3 * u      -> h' = silu(g) * PSUM_u = SX*SW3 * h
  h' quantized to fp8 hi+lo directly (|h'| < 240 by choice of SW3).
  PSUM_y = SX*SW3*SW2 * y  -> combine weight folded: cvec = c/(SX*SW3*SW2)
"""

import ml_dtypes
import numpy as np

import concourse.bass as bass  # noqa: F401  (engine plumbing)
import concourse.tile as tile
from concourse import bacc, mybir
from concourse.bass_utils import run_bass_kernel_spmd

T, H, F, E, TOPK = 4096, 1024, 512, 16, 4
NCORES = 8
F32 = mybir.dt.float32
BF16 = mybir.dt.bfloat16
FP8 = mybir.dt.float8e4
NPFP8 = ml_dtypes.float8_e4m3

SX, SW1, SW3, SW2 = 2.0, 64.0, 8.0, 64.0
SILU_SCALE = 1.0 / (SX * SW1)          # PSUM_g -> true g
CSCALE = 1.0 / (SX * SW3 * SW2)        # PSUM_y -> true y, folded into cvec

KP = H // 256    # stage-1 contraction k-pairs (DoubleRow: 256 per pair)
FPAIR = F // 256  # stage-2 contraction f-pairs

_nc_cache: dict = {}
LAST_CAPS = (1408, 1024)  # caps used by the most recent kernel() call


def _chunk_list(caps):
    """(slot, t0, tl) chunks of <=512 tokens covering both slots."""
    out = []
    t0 = 0
    for s in (0, 1):
        rem = caps[s]
        while rem > 0:
            tl = min(512, rem)
            out.append((s, t0, tl))
            t0 += tl
            rem -= tl
    return out


def _build_phase_b(caps: tuple[int, int]):
    """Expert FFN, fp8 DoubleRow with hi/lo error compensation.

    Inputs per core:
      w13q [2, 2, H, 2F]  per-slot, per-(hi,lo) hstack(w1[e].T*SW1, w3[e].T*SW3)
      w2q  [2, 2, F, H]   per-slot, per-(hi,lo) w2[e].T*SW2
      xgq  [2, H, CT]     per-(hi,lo) gathered tokens (transposed), fp8
      cvec [128, CT/128]  combine weight * CSCALE per gathered token
    Output:
      yg   [CT, H]        combine-weighted expert outputs, bf16
    """
    DR = mybir.MatmulPerfMode.DoubleRow
    SILU = mybir.ActivationFunctionType.Silu
    COPY = mybir.ActivationFunctionType.Copy
    CT = sum(caps)
    assert CT % 128 == 0
    nc = bacc.Bacc("TRN2", target_bir_lowering=False, debug=False,
                   num_devices=NCORES)
    w13q = nc.dram_tensor("w13q", [2, 2, H, 2 * F], FP8,
                          kind="ExternalInput").ap()
    w2q = nc.dram_tensor("w2q", [2, 2, F, H], FP8, kind="ExternalInput").ap()
    xgq = nc.dram_tensor("xgq", [2, H, CT], FP8, kind="ExternalInput").ap()
    cvec = nc.dram_tensor("cvec", [128, CT // 128], F32,
                          kind="ExternalInput").ap()
    yg = nc.dram_tensor("yg", [CT, H], BF16, kind="ExternalOutput").ap()

    chunks = _chunk_list(caps)

    with tile.TileContext(nc) as tc:
        with (
            tc.tile_pool(name="w13_p", bufs=1) as w13_p,
            tc.tile_pool(name="w2_p", bufs=1) as w2_p,
            tc.tile_pool(name="xg_p", bufs=2) as xg_p,
            tc.tile_pool(name="sg_p", bufs=2) as sg_p,
            tc.tile_pool(name="hp_p", bufs=2) as hp_p,
            tc.tile_pool(name="hq_p", bufs=2) as hq_p,
            tc.tile_pool(name="y_p", bufs=2) as y_p,
            tc.tile_pool(name="c_p", bufs=1) as c_p,
            tc.tile_pool(name="ps", bufs=8, space="PSUM") as ps_pool,
        ):
            c_sb = c_p.tile([128, CT // 128], F32)

            # Weights. w13 hi split per k-pair so the first matmuls only wait
            # on a 256KB DMA; lo and w2 arrive while hi*hi matmuls run.
            whi13 = [[w13_p.tile([128, 2, 2 * F], FP8, name=f"whi13_{s}_{kp}")
                      for kp in range(KP)] for s in range(2)]
            wlo13 = [[w13_p.tile([128, 2, 2 * F], FP8, name=f"wlo13_{s}_{kp}")
                      for kp in range(KP)] for s in range(2)]
            whi2 = [w2_p.tile([128, FPAIR, 2, H], FP8, name=f"whi2_{s}")
                    for s in range(2)]
            wlo2 = [w2_p.tile([128, FPAIR, 2, H], FP8, name=f"wlo2_{s}")
                    for s in range(2)]

            def load_w13(s, hi):
                tiles, v = (whi13, 0) if hi else (wlo13, 1)
                eng = nc.sync if hi else nc.gpsimd
                for kp in range(KP):
                    eng.dma_start(
                        tiles[s][kp][:],
                        w13q[s, v, kp * 256:(kp + 1) * 256, :].rearrange(
                            "(two p) f -> p two f", p=128))

            def load_w2(s):
                nc.gpsimd.dma_start(
                    whi2[s][:],
                    w2q[s, 0].rearrange("(fp two p) h -> p fp two h", p=128, two=2))
                nc.gpsimd.dma_start(
                    wlo2[s][:],
                    w2q[s, 1].rearrange("(fp two p) h -> p fp two h", p=128, two=2))

            # PE clock warm-up: dummy matmuls on a memset tile while the
            # first weight/activation DMAs stream in (the p-state model
            # upclocks after ~3us of continuous PE activity).
            warm = c_p.tile([128, 2, 128], FP8, name="warm")
            nc.gpsimd.memset(warm[:], 0)
            ps_warm = ps_pool.tile([128, 128], F32, tag="ps", name="ps_warm")
            for _ in range(24):
                nc.tensor.matmul(ps_warm[:], lhsT=warm[:], rhs=warm[:],
                                 start=True, stop=True, perf_mode=DR)

            load_w13(0, hi=True)

            xgq_r = [xgq[v].rearrange("(kp two p) t -> p kp two t", p=128, two=2)
                     for v in range(2)]

            def stage2(s, t0, tl, hq_hi, hq_lo, split_dma=False):
                y_sb = y_p.tile([128, 4, H], BF16, tag="y",
                                name=f"y_{t0}")
                for tt0 in range(0, tl, 128):
                    cidx = (t0 + tt0) // 128
                    for hh in range(2):
                        ps_y = ps_pool.tile([128, 512], F32, tag="ps",
                                            name=f"psy_{t0}_{tt0}_{hh}")
                        idx = 0
                        for ht, wt in ((hq_hi, whi2[s]), (hq_hi, wlo2[s]),
                                       (hq_lo, whi2[s])):
                            for fp in range(FPAIR):
                                nc.tensor.matmul(
                                    ps_y[:],
                                    lhsT=ht[:, fp, :, tt0:tt0 + 128],
                                    rhs=wt[:, fp, :, hh * 512:(hh + 1) * 512],
                                    start=(idx == 0),
                                    stop=(idx == 3 * FPAIR - 1),
                                    perf_mode=DR)
                                idx += 1
                        ydst = y_sb[:, tt0 // 128, hh * 512:(hh + 1) * 512]
                        if split_dma and hh == 1:
                            # last chunk: scale the two halves on different
                            # engines and ship each half immediately
                            nc.vector.tensor_scalar(
                                ydst, ps_y[:], c_sb[:, cidx:cidx + 1], None,
                                op0=mybir.AluOpType.mult)
                        else:
                            nc.scalar.activation(
                                ydst, ps_y[:], COPY,
                                scale=c_sb[:, cidx:cidx + 1])
                        if split_dma:
                            nc.sync.dma_start(
                                yg[t0 + tt0:t0 + tt0 + 128,
                                   hh * 512:(hh + 1) * 512], ydst)
                if not split_dma:
                    nc.sync.dma_start(
                        yg[t0:t0 + tl].rearrange("(n p) h -> p n h", p=128),
                        y_sb[:, :tl // 128])

            pending = None
            for ci, (s, t0, tl) in enumerate(chunks):
                xhi = xg_p.tile([128, KP, 2, 512], FP8, tag="xhi",
                                name=f"xhi_{ci}")
                xlo = xg_p.tile([128, KP, 2, 512], FP8, tag="xlo",
                                name=f"xlo_{ci}")
                if ci == 0:
                    # split the first activation load per k-pair so the very
                    # first matmul starts after ~2 small DMAs
                    for kp in range(KP):
                        nc.sync.dma_start(xhi[:, kp, :, :tl],
                                          xgq_r[0][:, kp, :, t0:t0 + tl])
                else:
                    nc.sync.dma_start(xhi[:, :, :, :tl],
                                      xgq_r[0][:, :, :, t0:t0 + tl])
                nc.scalar.dma_start(xlo[:, :, :, :tl],
                                    xgq_r[1][:, :, :, t0:t0 + tl])
                if ci == 0:
                    # remaining weights, behind chunk-0's activations in the
                    # DMA queues (the hi*hi products run first and only need
                    # whi13 + xhi)
                    load_w13(0, hi=False)
                    nc.gpsimd.dma_start(c_sb[:], cvec[:])
                    load_w2(0)
                elif ci == 1:
                    load_w13(1, hi=True)
                    load_w13(1, hi=False)
                    load_w2(1)

                hq_hi = hq_p.tile([128, FPAIR, 2, 512], FP8, tag="hqhi",
                                  name=f"hqhi_{ci}")
                hq_lo = hq_p.tile([128, FPAIR, 2, 512], FP8, tag="hqlo",
                                  name=f"hqlo_{ci}")
                ps_g = [ps_pool.tile([128, 512], F32, tag="ps",
                                     name=f"psg_{ci}_{fi}") for fi in range(4)]
                ps_u = [ps_pool.tile([128, 512], F32, tag="ps",
                                     name=f"psu_{ci}_{fi}") for fi in range(4)]

                def mm_s1(prod, fi, path, kp, first, last):
                    wt, xt = ((whi13[s], xhi), (whi13[s], xlo),
                              (wlo13[s], xhi))[prod]
                    ps = (ps_g, ps_u)[path][fi]
                    col0 = path * F + fi * 128
                    nc.tensor.matmul(
                        ps[:, :tl], lhsT=wt[kp][:, :, col0:col0 + 128],
                        rhs=xt[:, kp, :, :tl], start=first, stop=last,
                        perf_mode=DR)

                if ci == 0:
                    # hi*hi first across all groups (needs only whi13+xhi, so
                    # the PE runs while xlo/wlo13 stream in); then finish each
                    # group in turn so PSUM banks free progressively
                    for fi in range(4):
                        for path in range(2):
                            for kp in range(KP):
                                mm_s1(0, fi, path, kp, kp == 0, False)
                    for fi in range(4):
                        for path in range(2):
                            for prod in (1, 2):
                                for kp in range(KP):
                                    mm_s1(prod, fi, path, kp, False,
                                          prod == 2 and kp == KP - 1)
                else:
                    for fi in range(4):
                        for path in range(2):
                            idx = 0
                            for prod in range(3):
                                for kp in range(KP):
                                    mm_s1(prod, fi, path, kp, idx == 0,
                                          idx == 3 * KP - 1)
                                    idx += 1

                for fi in range(4):
                    fp, two = fi // 2, fi % 2
                    sg = sg_p.tile([128, 512], F32, tag="sg",
                                   name=f"sg_{ci}_{fi}")
                    nc.scalar.activation(sg[:, :tl], ps_g[fi][:, :tl], SILU,
                                         scale=SILU_SCALE)
                    hp = hp_p.tile([128, 512], F32, tag="hp",
                                   name=f"hp_{ci}_{fi}")
                    nc.vector.tensor_mul(hp[:, :tl], sg[:, :tl],
                                         ps_u[fi][:, :tl])
                    nc.vector.tensor_copy(hq_hi[:, fp, two, :tl], hp[:, :tl])
                    nc.vector.tensor_sub(hq_lo[:, fp, two, :tl], hp[:, :tl],
                                         hq_hi[:, fp, two, :tl])

                if pending is not None:
                    stage2(*pending)
                pending = (s, t0, tl, hq_hi, hq_lo)
            stage2(*pending, split_dma=True)

    nc.compile()
    return nc


def _phase_b_nc(caps):
    key = ("b", caps)
    if key not in _nc_cache:
        _nc_cache[key] = _build_phase_b(caps)
    return _nc_cache[key]


def _pad128(n: int) -> int:
    return max(128, (n + 127) // 128 * 128)


def _hilo(a: np.ndarray, scale: float):
    """fp8-e4m3 hi/lo decomposition of a*scale (hi + lo ~= a*scale)."""
    s = (a * scale).astype(np.float32)
    hi = s.astype(NPFP8)
    lo = (s - hi.astype(np.float32)).astype(NPFP8)
    return hi, lo


def kernel(hidden_states, gate_w, bias, w1, w3, w2):
    x = np.ascontiguousarray(np.asarray(hidden_states, dtype=np.float32))
    gate_w = np.asarray(gate_w, dtype=np.float32)
    bias = np.asarray(bias, dtype=np.float32)
    w1 = np.asarray(w1, dtype=np.float32)
    w3 = np.asarray(w3, dtype=np.float32)
    w2 = np.asarray(w2, dtype=np.float32)

    # ----

    # Routing on host, f32 (matches reference math; top-k ties -> lower idx).
    logits = x @ gate_w.T                               # [T, E]
    scores = 1.0 / (1.0 + np.exp(-logits))
    topi = np.argsort(-(scores + bias[None, :]), axis=1,
                      kind="stable")[:, :TOPK]          # [T, K]
    topw = np.take_along_axis(scores, topi, axis=1)
    topw = topw / topw.sum(axis=1, keepdims=True)
    combine = np.zeros((T, E), dtype=np.float32)
    np.put_along_axis(combine, topi, topw, axis=1)      # [T, E]

    # ---- Host dispatch: order experts by load, two slots per core ----
    idx_per_e = [np.nonzero(combine[:, e] > 0.0)[0] for e in range(E)]
    counts = np.array([len(ix) for ix in idx_per_e])
    order = np.argsort(-counts, kind="stable")
    slot0 = [int(order[c]) for c in range(NCORES)]
    slot1 = [int(order[NCORES + c]) for c in range(NCORES)]
    C0 = _pad128(int(counts[order[:NCORES]].max()))
    C1 = _pad128(int(counts[order[NCORES:]].max()))
    caps = (C0, C1)
    global LAST_CAPS
    LAST_CAPS = caps
    CT = C0 + C1

    # ---- Host quantization: fp8 hi/lo of activations and weights ----
    xT = np.ascontiguousarray(x.T)                      # [H, T]
    xhi, xlo = _hilo(xT, SX)

    in_maps = []
    for c in range(NCORES):
        pair = (slot0[c], slot1[c])
        idx_pad = np.zeros(CT, dtype=np.int64)
        cv = np.zeros(CT, dtype=np.float32)
        for s, e in enumerate(pair):
            off = s * C0
            ix = idx_per_e[e]
            idx_pad[off:off + len(ix)] = ix
            cv[off:off + len(ix)] = combine[ix, e] * CSCALE
        xgq = np.stack([np.ascontiguousarray(xhi[:, idx_pad]),
                        np.ascontiguousarray(xlo[:, idx_pad])])  # [2, H, CT]
        w13q = np.empty((2, 2, H, 2 * F), dtype=NPFP8)
        w2q = np.empty((2, 2, F, H), dtype=NPFP8)
        for s, e in enumerate(pair):
            w13 = np.concatenate([w1[e].T * SW1, w3[e].T * SW3], axis=1)
            hi, lo = _hilo(w13, 1.0)
            w13q[s, 0], w13q[s, 1] = hi, lo
            hi, lo = _hilo(w2[e].T, SW2)
            w2q[s, 0], w2q[s, 1] = hi, lo
        cv_tiled = np.ascontiguousarray(cv.reshape(CT // 128, 128).T)
        in_maps.append({"w13q": w13q, "w2q": w2q, "xgq": xgq,
                        "cvec": cv_tiled})

    # ---- Phase B: expert FFN on device (expert-parallel) ----
    ncB = _phase_b_nc(caps)
    resB = run_bass_kernel_spmd(ncB, in_maps, core_ids=list(range(NCORES)))

    # ---- Host combine: scatter-add in expert order ----
    out = np.zeros((T, H), dtype=np.float32)
    where = {}
    for c in range(NCORES):
        where[slot0[c]] = (c, 0)
        where[slot1[c]] = (c, C0)
    for e in range(E):
        c, off = where[e]
        ix = idx_per_e[e]
        if len(ix):
            out[ix] += resB.results[c]["yg"][off:off + len(ix)
                                             ].astype(np.float32)
    return out


# revision 19
# speedup vs baseline: 1.0374x; 1.0374x over previous
"""MiniMax-M2 MoE kernel for 8 Trainium2 NeuronCores.

Strategy (expert-parallel, sparse/routed):
  Host: router gate matmul + sigmoid + top-4 selection + combine-weight
    renormalization in f32 numpy (pure data movement / tiny matmul), then
    gather tokens per expert, pad each expert slot to a static capacity.
  Host: quantize activations and weights to fp8-e4m3 hi/lo pairs
    (value = hi + lo exactly to ~2^-14 relative) so the device can run all
    matmuls in DoubleRow fp8 perf mode while keeping ~bf16 accuracy via
    3-product error compensation (hi*hi + hi*lo + lo*hi; lo*lo dropped).
  Device (expert-parallel): per core, 2 expert slots' SwiGLU FFN over the
    gathered tokens; combine weight applied on device; bf16 outputs.
  Host: scatter-add per-expert outputs into the [T, H] result in expert
    order (matches the reference scan accumulation order).

Scale bookkeeping (per-tensor power-of-2 scales, folded into constants):
  x*SX, w1*SW1, w3*SW3, w2*SW2 quantized to fp8 hi+lo.
  PSUM_g = SX*SW1 * g      -> silu input scale 1/(SX*SW1)
  PSUM_u = SX*SW3 * u      -> h' = silu(g) * PSUM_u = SX*SW3 * h
  h' quantized to fp8 hi+lo directly (|h'| < 240 by choice of SW3).
  PSUM_y = SX*SW3*SW2 * y  -> combine weight folded: cvec = c/(SX*SW3*SW2)
"""

import ml_dtypes
import numpy as np

import concourse.bass as bass  # noqa: F401  (engine plumbing)
import concourse.tile as tile
from concourse import bacc, mybir
from concourse.bass_utils import run_bass_kernel_spmd

T, H, F, E, TOPK = 4096, 1024, 512, 16, 4
NCORES = 8
F32 = mybir.dt.float32
BF16 = mybir.dt.bfloat16
FP8 = mybir.dt.float8e4
NPFP8 = ml_dtypes.float8_e4m3

SX, SW1, SW3, SW2 = 2.0, 64.0, 8.0, 64.0
SILU_SCALE = 1.0 / (SX * SW1)          # PSUM_g -> true g
CSCALE = 1.0 / (SX * SW3 * SW2)        # PSUM_y -> true y, folded into cvec

KP = H // 256    # stage-1 contraction k-pairs (DoubleRow: 256 per pair)
FPAIR = F // 256  # stage-2 contraction f-pairs

_nc_cache: dict = {}
LAST_CAPS = (1408, 1024)  # caps used by the most recent kernel() call


def _chunk_list(caps):
    """(slot, t0, tl) chunks of <=512 tokens covering both slots.

    The smallest chunk is moved last so the kernel's drain tail (final
    stage-2 + output DMA) is as short as possible.
    """
    out = []
    t0 = 0
    for s in (0, 1):
        rem = caps[s]
        while rem > 0:
            tl = min(512, rem)
            out.append((s, t0, tl))
            t0 += tl
            rem -= tl
    smallest = min(range(len(out)), key=lambda i: out[i][2])
    out.append(out.pop(smallest))
    return out


def _build_phase_b(caps: tuple[int, int]):
    """Expert FFN, fp8 DoubleRow with hi/lo error compensation.

    Inputs per core:
      w13q [2, 2, H, 2F]  per-slot, per-(hi,lo) hstack(w1[e].T*SW1, w3[e].T*SW3)
      w2q  [2, 2, F, H]   per-slot, per-(hi,lo) w2[e].T*SW2
      xgq  [2, H, CT]     per-(hi,lo) gathered tokens (transposed), fp8
      cvec [128, CT/128]  combine weight * CSCALE per gathered token
    Output:
      yg   [CT, H]        combine-weighted expert outputs, bf16
    """
    DR = mybir.MatmulPerfMode.DoubleRow
    SILU = mybir.ActivationFunctionType.Silu
    COPY = mybir.ActivationFunctionType.Copy
    CT = sum(caps)
    assert CT % 128 == 0
    nc = bacc.Bacc("TRN2", target_bir_lowering=False, debug=False,
                   num_devices=NCORES)
    w13q = nc.dram_tensor("w13q", [2, 2, H, 2 * F], FP8,
                          kind="ExternalInput").ap()
    w2q = nc.dram_tensor("w2q", [2, 2, F, H], FP8, kind="ExternalInput").ap()
    xgq = nc.dram_tensor("xgq", [2, H, CT], FP8, kind="ExternalInput").ap()
    cvec = nc.dram_tensor("cvec", [128, CT // 128], F32,
                          kind="ExternalInput").ap()
    yg = nc.dram_tensor("yg", [CT, H], BF16, kind="ExternalOutput").ap()

    chunks = _chunk_list(caps)

    with tile.TileContext(nc) as tc:
        with (
            tc.tile_pool(name="w13_p", bufs=1) as w13_p,
            tc.tile_pool(name="w2_p", bufs=1) as w2_p,
            tc.tile_pool(name="xg_p", bufs=2) as xg_p,
            tc.tile_pool(name="sg_p", bufs=2) as sg_p,
            tc.tile_pool(name="hp_p", bufs=2) as hp_p,
            tc.tile_pool(name="hq_p", bufs=2) as hq_p,
            tc.tile_pool(name="y_p", bufs=2) as y_p,
            tc.tile_pool(name="c_p", bufs=1) as c_p,
            tc.tile_pool(name="ps", bufs=8, space="PSUM") as ps_pool,
        ):
            c_sb = c_p.tile([128, CT // 128], F32)

            # Weights. w13 hi split per k-pair so the first matmuls only wait
            # on a 256KB DMA; lo and w2 arrive while hi*hi matmuls run.
            whi13 = [[w13_p.tile([128, 2, 2 * F], FP8, name=f"whi13_{s}_{kp}")
                      for kp in range(KP)] for s in range(2)]
            wlo13 = [[w13_p.tile([128, 2, 2 * F], FP8, name=f"wlo13_{s}_{kp}")
                      for kp in range(KP)] for s in range(2)]
            whi2 = [w2_p.tile([128, FPAIR, 2, H], FP8, name=f"whi2_{s}")
                    for s in range(2)]
            wlo2 = [w2_p.tile([128, FPAIR, 2, H], FP8, name=f"wlo2_{s}")
                    for s in range(2)]

            def load_w13(s, hi):
                tiles, v = (whi13, 0) if hi else (wlo13, 1)
                eng = nc.sync if hi else nc.gpsimd
                for kp in range(KP):
                    eng.dma_start(
                        tiles[s][kp][:],
                        w13q[s, v, kp * 256:(kp + 1) * 256, :].rearrange(
                            "(two p) f -> p two f", p=128))

            def load_w2(s):
                nc.gpsimd.dma_start(
                    whi2[s][:],
                    w2q[s, 0].rearrange("(fp two p) h -> p fp two h", p=128, two=2))
                nc.gpsimd.dma_start(
                    wlo2[s][:],
                    w2q[s, 1].rearrange("(fp two p) h -> p fp two h", p=128, two=2))

            # PE clock warm-up: dummy matmuls on a memset tile while the
            # first weight/activation DMAs stream in (the p-state model
            # upclocks after ~3us of continuous PE activity).
            warm = c_p.tile([128, 2, 128], FP8, name="warm")
            nc.gpsimd.memset(warm[:], 0)
            ps_warm = ps_pool.tile([128, 128], F32, tag="ps", name="ps_warm")
            for _ in range(24):
                nc.tensor.matmul(ps_warm[:], lhsT=warm[:], rhs=warm[:],
                                 start=True, stop=True, perf_mode=DR)



            xgq_r = [xgq[v].rearrange("(kp two p) t -> p kp two t", p=128, two=2)
                     for v in range(2)]

            def stage2(s, t0, tl, hq_hi, hq_lo, split_dma=False):
                y_sb = y_p.tile([128, 4, H], BF16, tag="y",
                                name=f"y_{t0}")
                for tt0 in range(0, tl, 128):
                    cidx = (t0 + tt0) // 128
                    for hh in range(2):
                        ps_y = ps_pool.tile([128, 512], F32, tag="ps",
                                            name=f"psy_{t0}_{tt0}_{hh}")
                        idx = 0
                        for ht, wt in ((hq_hi, whi2[s]), (hq_hi, wlo2[s]),
                                       (hq_lo, whi2[s])):
                            for fp in range(FPAIR):
                                nc.tensor.matmul(
                                    ps_y[:],
                                    lhsT=ht[:, fp, :, tt0:tt0 + 128],
                                    rhs=wt[:, fp, :, hh * 512:(hh + 1) * 512],
                                    start=(idx == 0),
                                    stop=(idx == 3 * FPAIR - 1),
                                    perf_mode=DR)
                                idx += 1
                        ydst = y_sb[:, tt0 // 128, hh * 512:(hh + 1) * 512]
                        if split_dma and hh == 1:
                            # last chunk: scale the two halves on different
                            # engines so the tile ships sooner
                            nc.vector.tensor_scalar(
                                ydst, ps_y[:], c_sb[:, cidx:cidx + 1], None,
                                op0=mybir.AluOpType.mult)
                        else:
                            nc.scalar.activation(
                                ydst, ps_y[:], COPY,
                                scale=c_sb[:, cidx:cidx + 1])
                    if split_dma:
                        nc.sync.dma_start(
                            yg[t0 + tt0:t0 + tt0 + 128],
                            y_sb[:, tt0 // 128])
                if not split_dma:
                    nc.sync.dma_start(
                        yg[t0:t0 + tl].rearrange("(n p) h -> p n h", p=128),
                        y_sb[:, :tl // 128])

            pending = None
            for ci, (s, t0, tl) in enumerate(chunks):
                xhi = xg_p.tile([128, KP, 2, 512], FP8, tag="xhi",
                                name=f"xhi_{ci}")
                xlo = xg_p.tile([128, KP, 2, 512], FP8, tag="xlo",
                                name=f"xlo_{ci}")
                if ci == 0:
                    # interleave the per-k-pair weight and activation pieces
                    # so the k-th hi*hi matmul wave starts after ~2 small
                    # DMAs instead of after the full weight matrix
                    for kp in range(KP):
                        nc.sync.dma_start(
                            whi13[s][kp][:],
                            w13q[s, 0, kp * 256:(kp + 1) * 256, :].rearrange(
                                "(two p) f -> p two f", p=128))
                        nc.sync.dma_start(xhi[:, kp, :, :tl],
                                          xgq_r[0][:, kp, :, t0:t0 + tl])
                else:
                    nc.sync.dma_start(xhi[:, :, :, :tl],
                                      xgq_r[0][:, :, :, t0:t0 + tl])
                nc.scalar.dma_start(xlo[:, :, :, :tl],
                                    xgq_r[1][:, :, :, t0:t0 + tl])
                if ci == 0:
                    # remaining weights, behind chunk-0's activations in the
                    # DMA queues (the hi*hi products run first and only need
                    # whi13 + xhi)
                    load_w13(0, hi=False)
                    nc.gpsimd.dma_start(c_sb[:], cvec[:])
                    load_w2(0)
                elif ci == 1:
                    load_w13(1, hi=True)
                    load_w13(1, hi=False)
                    load_w2(1)

                hq_hi = hq_p.tile([128, FPAIR, 2, 512], FP8, tag="hqhi",
                                  name=f"hqhi_{ci}")
                hq_lo = hq_p.tile([128, FPAIR, 2, 512], FP8, tag="hqlo",
                                  name=f"hqlo_{ci}")
                ps_g = [ps_pool.tile([128, 512], F32, tag="ps",
                                     name=f"psg_{ci}_{fi}") for fi in range(4)]
                ps_u = [ps_pool.tile([128, 512], F32, tag="ps",
                                     name=f"psu_{ci}_{fi}") for fi in range(4)]

                def mm_s1(prod, fi, path, kp, first, last):
                    wt, xt = ((whi13[s], xhi), (whi13[s], xlo),
                              (wlo13[s], xhi))[prod]
                    ps = (ps_g, ps_u)[path][fi]
                    col0 = path * F + fi * 128
                    nc.tensor.matmul(
                        ps[:, :tl], lhsT=wt[kp][:, :, col0:col0 + 128],
                        rhs=xt[:, kp, :, :tl], start=first, stop=last,
                        perf_mode=DR)

                if ci == 0:
                    # hi*hi first across all groups, kp-major (each kp wave
                    # needs only one small weight+activation DMA pair); then
                    # finish each group in turn so PSUM banks free
                    # progressively
                    for kp in range(KP):
                        for fi in range(4):
                            for path in range(2):
                                mm_s1(0, fi, path, kp, kp == 0, False)
                    for fi in range(4):
                        for path in range(2):
                            for prod in (1, 2):
                                for kp in range(KP):
                                    mm_s1(prod, fi, path, kp, False,
                                          prod == 2 and kp == KP - 1)
                else:
                    for fi in range(4):
                        for path in range(2):
                            idx = 0
                            for prod in range(3):
                                for kp in range(KP):
                                    mm_s1(prod, fi, path, kp, idx == 0,
                                          idx == 3 * KP - 1)
                                    idx += 1

                for fi in range(4):
                    fp, two = fi // 2, fi % 2
                    sg = sg_p.tile([128, 512], F32, tag="sg",
                                   name=f"sg_{ci}_{fi}")
                    nc.scalar.activation(sg[:, :tl], ps_g[fi][:, :tl], SILU,
                                         scale=SILU_SCALE)
                    hp = hp_p.tile([128, 512], F32, tag="hp",
                                   name=f"hp_{ci}_{fi}")
                    nc.vector.tensor_mul(hp[:, :tl], sg[:, :tl],
                                         ps_u[fi][:, :tl])
                    nc.vector.tensor_copy(hq_hi[:, fp, two, :tl], hp[:, :tl])
                    nc.vector.tensor_sub(hq_lo[:, fp, two, :tl], hp[:, :tl],
                                         hq_hi[:, fp, two, :tl])

                if pending is not None:
                    stage2(*pending)
                pending = (s, t0, tl, hq_hi, hq_lo)
            stage2(*pending, split_dma=True)

    nc.compile()
    return nc


def _phase_b_nc(caps):
    key = ("b", caps)
    if key not in _nc_cache:
        _nc_cache[key] = _build_phase_b(caps)
    return _nc_cache[key]


def _pad128(n: int) -> int:
    return max(128, (n + 127) // 128 * 128)


def _hilo(a: np.ndarray, scale: float):
    """fp8-e4m3 hi/lo decomposition of a*scale (hi + lo ~= a*scale)."""
    s = (a * scale).astype(np.float32)
    hi = s.astype(NPFP8)
    lo = (s - hi.astype(np.float32)).astype(NPFP8)
    return hi, lo


def kernel(hidden_states, gate_w, bias, w1, w3, w2):
    x = np.ascontiguousarray(np.asarray(hidden_states, dtype=np.float32))
    gate_w = np.asarray(gate_w, dtype=np.float32)
    bias = np.asarray(bias, dtype=np.float32)
    w1 = np.asarray(w1, dtype=np.float32)
    w3 = np.asarray(w3, dtype=np.float32)
    w2 = np.asarray(w2, dtype=np.float32)

    # ----

    # Routing on host, f32 (matches reference math; top-k ties -> lower idx).
    logits = x @ gate_w.T                               # [T, E]
    scores = 1.0 / (1.0 + np.exp(-logits))
    topi = np.argsort(-(scores + bias[None, :]), axis=1,
                      kind="stable")[:, :TOPK]          # [T, K]
    topw = np.take_along_axis(scores, topi, axis=1)
    topw = topw / topw.sum(axis=1, keepdims=True)
    combine = np.zeros((T, E), dtype=np.float32)
    np.put_along_axis(combine, topi, topw, axis=1)      # [T, E]

    # ---- Host dispatch: order experts by load, two slots per core ----
    idx_per_e = [np.nonzero(combine[:, e] > 0.0)[0] for e in range(E)]
    counts = np.array([len(ix) for ix in idx_per_e])
    order = np.argsort(-counts, kind="stable")
    slot0 = [int(order[c]) for c in range(NCORES)]
    slot1 = [int(order[NCORES + c]) for c in range(NCORES)]
    C0 = _pad128(int(counts[order[:NCORES]].max()))
    C1 = _pad128(int(counts[order[NCORES:]].max()))
    caps = (C0, C1)
    global LAST_CAPS
    LAST_CAPS = caps
    CT = C0 + C1

    # ---- Host quantization: fp8 hi/lo of activations and weights ----
    xT = np.ascontiguousarray(x.T)                      # [H, T]
    xhi, xlo = _hilo(xT, SX)

    in_maps = []
    for c in range(NCORES):
        pair = (slot0[c], slot1[c])
        idx_pad = np.zeros(CT, dtype=np.int64)
        cv = np.zeros(CT, dtype=np.float32)
        for s, e in enumerate(pair):
            off = s * C0
            ix = idx_per_e[e]
            idx_pad[off:off + len(ix)] = ix
            cv[off:off + len(ix)] = combine[ix, e] * CSCALE
        xgq = np.stack([np.ascontiguousarray(xhi[:, idx_pad]),
                        np.ascontiguousarray(xlo[:, idx_pad])])  # [2, H, CT]
        w13q = np.empty((2, 2, H, 2 * F), dtype=NPFP8)
        w2q = np.empty((2, 2, F, H), dtype=NPFP8)
        for s, e in enumerate(pair):
            w13 = np.concatenate([w1[e].T * SW1, w3[e].T * SW3], axis=1)
            hi, lo = _hilo(w13, 1.0)
            w13q[s, 0], w13q[s, 1] = hi, lo
            hi, lo = _hilo(w2[e].T, SW2)
            w2q[s, 0], w2q[s, 1] = hi, lo
        cv_tiled = np.ascontiguousarray(cv.reshape(CT // 128, 128).T)
        in_maps.append({"w13q": w13q, "w2q": w2q, "xgq": xgq,
                        "cvec": cv_tiled})

    # ---- Phase B: expert FFN on device (expert-parallel) ----
    ncB = _phase_b_nc(caps)
    resB = run_bass_kernel_spmd(ncB, in_maps, core_ids=list(range(NCORES)))

    # ---- Host combine: scatter-add in expert order ----
    out = np.zeros((T, H), dtype=np.float32)
    where = {}
    for c in range(NCORES):
        where[slot0[c]] = (c, 0)
        where[slot1[c]] = (c, C0)
    for e in range(E):
        c, off = where[e]
        ix = idx_per_e[e]
        if len(ix):
            out[ix] += resB.results[c]["yg"][off:off + len(ix)
                                             ].astype(np.float32)
    return out
